# revision 11
# baseline (speedup 1.0000x reference)
"""Trainium2 Bass kernel for the Boat Dynamic System problem.

Math: out[b, c] = sum_f V[b, f] * coeffs[c, f] where V = [base, pro*base,
rud*base] and base = 15 quadratic monomials of s = (u, v, r, Pf).

Folding pro/rud (scalars picked from cmd on the host) gives an effective
[4, 15] coefficient matrix, i.e. out_c = s~^T Q_c s~ with s~ = (1, u, v, r, Pf).

J8 path (default): only the 4-dim span {Q_c} must be expressed, so a
Levenberg-Marquardt fit on the host finds J=8 vectors w_j in R^5 and
lam [4, 8] with Q_c = sum_j lam_cj w_j w_j^T (72 unknowns >= 60 equations;
exact for generic inputs, residual checked, falls back to the J=15 path).

Device pipeline per [128, 512] tile (16384 batch elements, per core):
  1. contiguous DMA of state (natural layout)
  2. PE transposes of four [128, 128] blocks -> partition q = 4n+f
     (n = batch-sub 0..31, f = component), column = batch-chunk
  3. DVE evac PSUM -> SBUF
  4. M1: two row-tiled K=64 matmuls (strips at tile rows 0/64) project the
     16 records per strip-column onto the 8 w_j -> Y [128, 1024] PSUM
  5. ACT Square(Y + bias) -> fsb [128, 1024] SBUF (bias = w_j0 constant)
  6. M2 fused with the output transpose: for each 128-col block of fsb,
     matmul(lhsT=fsb_block, rhs=lamblk [128, 64]) emits [128, 64] PSUM that
     is already batch-natural (partition = record-group, free = 16*(rec,c))
  7. DVE evac, contiguous DMA out

Matmul dtype is fp32r (full-rate; tf32-like rounding) or fp32 (quarter-rate,
exact) via BOAT_PRECISE=1. BOAT_J15=1 forces the legacy 15-square pipeline.
"""

import os

import numpy as np

NCORES = 8
B = 2097152
BS = B // NCORES          # 262144 rows per core
DT = 0.01
NTILES = 16               # tiles per core
TILE_B = BS // NTILES     # 16384 batch elements per tile
NCOL = 512                # columns per tile (32 batch elements per column)

_PAIRS = [(a, b) for a in range(5) for b in range(a, 5)]  # 15 (a<=b) pairs
_MONO2FEAT = {
    (0, 0): 0, (0, 1): 1, (0, 2): 2, (0, 3): 3, (0, 4): 4,
    (1, 1): 5, (1, 2): 6, (1, 3): 7, (1, 4): 8,
    (2, 2): 9, (2, 3): 10, (2, 4): 11,
    (3, 3): 12, (3, 4): 13,
    (4, 4): 14,
}

_NC_CACHE = {}
LAST_RESULT = [None]


# ---------------------------------------------------------------------------
# host math: effective quadratic forms and the J=8 decomposition
# ---------------------------------------------------------------------------

def _build_Q(t, cmd, coeffs):
    """Q [4, 5, 5] symmetric with out_c = s~^T Q_c s~, s~ = (1, u, v, r, Pf)."""
    idx = int(np.round(float(np.asarray(t).reshape(-1)[0]) / DT))
    pro = float(cmd[idx, 0])
    rud = float(cmd[idx, 1])
    cf = np.asarray(coeffs, dtype=np.float64)
    ceff = cf[:, 0:15] + pro * cf[:, 15:30] + rud * cf[:, 30:45]  # [4, 15]
    Q = np.zeros((4, 5, 5))
    for m, (x, y) in enumerate(_PAIRS):
        g = ceff[:, _MONO2FEAT[(x, y)]]
        if x == y:
            Q[:, x, y] += g
        else:
            Q[:, x, y] += g / 2
            Q[:, y, x] += g / 2
    return Q


def _j8_residual(W, lam, Q):
    M = np.einsum('ja,jb->jab', W, W)
    fit = np.einsum('cj,jab->cab', lam, M)
    R = Q - fit
    iu = np.triu_indices(5)
    return R[:, iu[0], iu[1]].ravel()


def _j8_jac(W, lam):
    J, D = W.shape
    C = lam.shape[0]
    iu = np.triu_indices(D)
    Jm = np.zeros((C * len(iu[0]), J * D + C * J))
    for c in range(C):
        for k, (a, b) in enumerate(zip(*iu)):
            r = c * 15 + k
            for j in range(J):
                Jm[r, J * D + c * J + j] = -W[j, a] * W[j, b]
                Jm[r, j * D + a] += -lam[c, j] * W[j, b]
                Jm[r, j * D + b] += -lam[c, j] * W[j, a]
    return Jm


def _solve_j8(Q, J=8, iters=250):
    """LM with restarts; returns (resid, amp, W [8,5], lam [4,8])."""
    rng = np.random.default_rng(0)
    best = None
    for trial in range(16):
        W = rng.normal(size=(J, 5))
        lam = rng.normal(size=(4, J)) * 0.3
        mu = 1e-3
        for _ in range(iters):
            r = _j8_residual(W, lam, Q)
            f = r @ r
            Jm = _j8_jac(W, lam)
            H = Jm.T @ Jm + mu * np.eye(Jm.shape[1])
            try:
                step = np.linalg.solve(H, Jm.T @ r)
            except np.linalg.LinAlgError:
                break
            Wn = W - step[:J * 5].reshape(J, 5)
            ln = lam - step[J * 5:].reshape(4, J)
            rn = _j8_residual(Wn, ln, Q)
            if rn @ rn < f:
                W, lam = Wn, ln
                mu = max(mu * 0.5, 1e-12)
                if rn @ rn < 1e-24:
                    break
            else:
                mu *= 4.0
                if mu > 1e12:
                    break
        r = _j8_residual(W, lam, Q)
        f = float(np.sqrt(r @ r))
        s = np.linalg.norm(W, axis=1, keepdims=True)
        s[s == 0] = 1
        W2 = W / s
        lam2 = lam * (s.ravel() ** 2)[None, :]
        amp = float(np.abs(lam2).sum())
        if best is None or (f, amp) < (best[0], best[1]):
            best = (f, amp, W2, lam2)
        if f < 1e-10 and amp < 60:
            break
    return best


def _host_weights_j8(W, lam):
    """wexp [128, 128], biasv [128, 1], lamblk [128, 64] device constants."""
    wexp = np.zeros((128, 128), dtype=np.float32)
    for s in range(2):
        for v in range(16):
            for f in range(4):
                for j in range(8):
                    wexp[64 * s + 4 * v + f, 8 * v + j] = W[j, 1 + f]
    biasv = np.zeros((128, 1), dtype=np.float32)
    for v in range(16):
        for j in range(8):
            biasv[8 * v + j, 0] = W[j, 0]
    lamblk = np.zeros((128, 64), dtype=np.float32)
    for v in range(16):
        for j in range(8):
            for c in range(4):
                lamblk[8 * v + j, 4 * v + c] = lam[c, j]
    return wexp, biasv, lamblk


def _build_nc_j8(precise: bool):
    """fp16 pipeline: DMA-xbar input transpose, fp16 matmuls, fp16 output.

    fp16 inputs carry a 10-bit mantissa -- the same effective precision as
    fp32r matmuls -- but run at full PE rate with fast weight load, and
    2-byte dtypes unlock the DMA transpose engine (input lands f-major in
    SBUF with no PE/DVE work) and halve both DMA directions.
    """
    import concourse.bacc as bacc
    import concourse.mybir as mybir
    import concourse.tile as tile

    nc = bacc.Bacc("TRN2", target_bir_lowering=False, debug=False)
    f32 = mybir.dt.float32
    f16 = mybir.dt.float16
    Square = mybir.ActivationFunctionType.Square

    state = nc.dram_tensor("state", [BS, 4], f16, kind="ExternalInput")
    wexp_d = nc.dram_tensor("wexp", [128, 128], f16, kind="ExternalInput")
    biasv_d = nc.dram_tensor("biasv", [128, 1], f32, kind="ExternalInput")
    lam_d = nc.dram_tensor("lam", [128, 64], f16, kind="ExternalInput")
    out = nc.dram_tensor("out", [BS, 4], f16, kind="ExternalOutput")

    NMEGA = 4                  # DMA granularity: 4 compute-tiles per transfer
    MT = NTILES // NMEGA       # 4 compute-tiles per mega
    MCOL = MT * NCOL           # 2048 ssb columns per mega

    # per mega: X [2048, 128] with row c2 = one record-group of 32 records
    # (128 contiguous fp16); the xbar transpose lands it f-major in SBUF.
    state_r = state[:, :].rearrange(
        "(M c2 n) f -> M c2 (n f)", M=NMEGA, c2=MCOL, n=32
    )
    # stride-4 M2 slicing puts 128 consecutive records on each partition:
    # osb[p, 4*n + c] = out[M*65536 + T*16384 + 128*p + n, c]  (1KB chunks)
    out_r = out[:, :].rearrange(
        "(M T p n) f -> M p T n f", M=NMEGA, T=MT, p=128, n=128
    )

    with tile.TileContext(nc) as tc:
        with (
            tc.tile_pool(name="consts", bufs=1) as cpool,
            tc.tile_pool(name="si", bufs=3) as si,
            tc.tile_pool(name="so", bufs=2) as so,
            tc.tile_pool(name="fs", bufs=6) as fs,
            tc.tile_pool(name="ps", bufs=2, space="PSUM") as ps,
            tc.tile_pool(name="po", bufs=3, space="PSUM") as po,
        ):
            # PE warmup: dependency-free matmuls run during the DMA prefetch
            # window so HAM un-throttles the PE clock before real work lands
            wdum = cpool.tile([128, 128], f16)
            nc.gpsimd.memset(wdum[:], 0.0)
            rdum = cpool.tile([128, NCOL], f16)
            nc.gpsimd.memset(rdum[:], 0.0)
            pdum = ps.tile([128, NCOL], f32, tag="warm", bufs=1)
            for i in range(16):
                nc.tensor.matmul(
                    out=pdum[:],
                    lhsT=wdum[:],
                    rhs=rdum[:],
                    start=True,
                    stop=True,
                    tile_position=(0, 0),
                    skip_group_check=True,
                )

            # input transposes first in program order: the sync HWDGE queue
            # starts streaming them immediately; consts go on the scalar queue
            ssb_t = []
            for M in range(NMEGA):
                # xbar: ssb[4n+f, c2] = state[M*65536 + 32*c2 + n, f]
                ssb = si.tile([128, MCOL], f16, name=f"ssb{M}", tag="ssb")
                if M == 0:
                    # small lead chunk so the first compute tile starts early
                    nc.sync.dma_start(
                        out=ssb[:, 0:NCOL], in_=state_r[0, 0:NCOL],
                        transpose=True,
                    )
                    nc.sync.dma_start(
                        out=ssb[:, NCOL:MCOL], in_=state_r[0, NCOL:MCOL],
                        transpose=True,
                    )
                else:
                    nc.sync.dma_start(
                        out=ssb[:], in_=state_r[M], transpose=True
                    )
                ssb_t.append(ssb)

            wexp_sb = cpool.tile([128, 128], f16)
            nc.scalar.dma_start(out=wexp_sb[:], in_=wexp_d[:, :])
            biasv_sb = cpool.tile([128, 1], f32)
            nc.scalar.dma_start(out=biasv_sb[:], in_=biasv_d[:, :])
            lam_sb = cpool.tile([128, 64], f16)
            nc.scalar.dma_start(out=lam_sb[:], in_=lam_d[:, :])

            for M in range(NMEGA):
                ssb = ssb_t[M]
                osb = so.tile([128, MCOL], f16)

                for c in range(MT):
                    sl = slice(c * NCOL, (c + 1) * NCOL)
                    # M1: 2 row-tiled K=64 strips -> Y [128, 1024] (2 banks)
                    yps = ps.tile([128, 2 * NCOL], f32)
                    for s in range(2):
                        nc.tensor.matmul(
                            out=yps[:, s * NCOL:(s + 1) * NCOL],
                            lhsT=wexp_sb[64 * s:64 * (s + 1), :],
                            rhs=ssb[64 * s:64 * (s + 1), sl],
                            start=True,
                            stop=True,
                            tile_position=(64 * s, 0),
                        )

                    # squares (bias folds the w_j0 constant component)
                    fsb = fs.tile([128, 2 * NCOL], f16)
                    nc.scalar.activation(
                        out=fsb[:],
                        in_=yps[:],
                        func=Square,
                        bias=biasv_sb[:, 0:1],
                        scale=1.0,
                    )

                    # M2 fused with output transpose: lhsT = squared
                    # features (stride-4 cols: partition m = group 4m+phi),
                    # rhs = block-diag lambda; out batch-natural, 1KB chunks.
                    ops = po.tile([128, NCOL], f32)
                    for s in range(2):
                        for phi in range(4):
                            c0 = 128 * phi + 64 * s
                            nc.tensor.matmul(
                                out=ops[:, c0:c0 + 64],
                                lhsT=fsb[:, NCOL * s + phi:NCOL * (s + 1):4],
                                rhs=lam_sb[:],
                                start=True,
                                stop=True,
                                tile_position=(0, 0),
                            )
                    nc.vector.tensor_copy(out=osb[:, sl], in_=ops[:])
                nc.sync.dma_start(out=out_r[M], in_=osb[:])

    nc.finalize()
    return nc


# ---------------------------------------------------------------------------
# legacy J=15 pipeline (fallback when the J=8 fit fails)
# ---------------------------------------------------------------------------

def _square_basis():
    """15 fixed vectors w_j in R^5 whose squared functionals span quadratics."""
    W5 = np.zeros((15, 5), dtype=np.float64)
    for j, (a, b) in enumerate(_PAIRS):
        W5[j, a] += 1.0
        if b != a:
            W5[j, b] += 1.0
    M = np.zeros((15, 15), dtype=np.float64)
    for m, (x, y) in enumerate(_PAIRS):
        for j in range(15):
            M[m, j] = W5[j, x] * W5[j, y] * (1.0 if x == y else 2.0)
    return W5, M


def _host_weights_j15(t, cmd, coeffs):
    """Fold cmd/coeffs into the device weight tensors (all tiny)."""
    idx = int(np.round(float(np.asarray(t).reshape(-1)[0]) / DT))
    pro = float(cmd[idx, 0])
    rud = float(cmd[idx, 1])
    cf = np.asarray(coeffs, dtype=np.float64)
    ceff = cf[:, 0:15] + pro * cf[:, 15:30] + rud * cf[:, 30:45]  # [4, 15]

    gamma = np.zeros((4, 15), dtype=np.float64)
    for m, (x, y) in enumerate(_PAIRS):
        gamma[:, m] = ceff[:, _MONO2FEAT[(x, y)]]

    W5, M = _square_basis()
    lam45 = np.linalg.solve(M, gamma.T).T  # [4, 15]

    wexp4 = np.zeros((128, 120), dtype=np.float32)
    for t_ in range(4):
        for g in range(8):
            for j in range(15):
                for f in range(4):
                    wexp4[32 * t_ + 4 * g + f, g * 15 + j] = W5[j, 1 + f]

    biasw = np.zeros((120, 1), dtype=np.float32)
    for g in range(8):
        for j in range(15):
            biasw[g * 15 + j, 0] = W5[j, 0]

    lam = np.zeros((120, 32), dtype=np.float32)
    for g in range(8):
        for j in range(15):
            for c in range(4):
                lam[g * 15 + j, 4 * g + c] = lam45[c, j]

    lamAB = np.zeros((120, 128), dtype=np.float32)
    lamAB[:, 0:32] = lam
    lamAB[:, 96:128] = lam

    return wexp4, biasw, lamAB


def _build_nc_j15(precise: bool):
    import concourse.bacc as bacc
    import concourse.mybir as mybir
    import concourse.tile as tile
    from concourse.masks import make_identity

    nc = bacc.Bacc("TRN2", target_bir_lowering=False, debug=False)
    f32 = mybir.dt.float32
    mmdt = f32 if precise else mybir.dt.float32r
    Square = mybir.ActivationFunctionType.Square

    state = nc.dram_tensor("state", [BS, 4], mmdt, kind="ExternalInput")
    wexp_d = nc.dram_tensor("wexp", [128, 120], mmdt, kind="ExternalInput")
    biasw_d = nc.dram_tensor("biasw", [120, 1], f32, kind="ExternalInput")
    lam_d = nc.dram_tensor("lam", [120, 128], mmdt, kind="ExternalInput")
    out = nc.dram_tensor("out", [BS, 4], f32, kind="ExternalOutput")

    state_r = state[:, :].rearrange(
        "(T blk p n) f -> T p blk n f", T=NTILES, blk=4, p=128, n=32
    )
    out_r = out[:, :].rearrange(
        "(T blk p n) f -> T p blk n f", T=NTILES, blk=4, p=128, n=32
    )

    with tile.TileContext(nc) as tc:
        with (
            tc.tile_pool(name="consts", bufs=1) as cpool,
            tc.tile_pool(name="sb", bufs=4) as sb,
            tc.tile_pool(name="ps", bufs=1, space="PSUM") as ps,
        ):
            ident = cpool.tile([128, 128], f32)
            make_identity(nc, ident[:])
            identr = cpool.tile([128, 128], mmdt)
            nc.vector.tensor_copy(out=identr[:], in_=ident[:])
            wexp_sb = cpool.tile([128, 120], mmdt)
            nc.sync.dma_start(out=wexp_sb[:], in_=wexp_d[:, :])
            biasw_sb = cpool.tile([120, 1], f32)
            nc.sync.dma_start(out=biasw_sb[:], in_=biasw_d[:, :])
            lam_sb = cpool.tile([120, 128], mmdt)
            nc.sync.dma_start(out=lam_sb[:], in_=lam_d[:, :])

            for T in range(NTILES):
                xn = sb.tile([128, NCOL], mmdt)
                nc.sync.dma_start(out=xn[:], in_=state_r[T])

                spsum = ps.tile([128, NCOL], mmdt)
                for blk in range(4):
                    nc.tensor.transpose(
                        out=spsum[:, blk * 128:(blk + 1) * 128],
                        in_=xn[:, blk * 128:(blk + 1) * 128],
                        identity=identr[:],
                    )
                ssb = sb.tile([128, NCOL], mmdt)
                nc.vector.tensor_copy(out=ssb[:], in_=spsum[:])

                fsb = sb.tile([120, 4 * NCOL], mmdt)
                for h in range(2):
                    yps = ps.tile([120, 2 * NCOL], f32, tag=f"y{h}")
                    for u in range(2):
                        t_ = 2 * h + u
                        nc.tensor.matmul(
                            out=yps[:, u * NCOL:(u + 1) * NCOL],
                            lhsT=wexp_sb[32 * t_:32 * (t_ + 1), :],
                            rhs=ssb[32 * t_:32 * (t_ + 1), :],
                            start=True,
                            stop=True,
                            tile_position=(32 * t_, 0),
                        )
                    nc.scalar.activation(
                        out=fsb[:, h * 2 * NCOL:(h + 1) * 2 * NCOL],
                        in_=yps[:],
                        func=Square,
                        bias=biasw_sb[:, 0:1],
                        scale=1.0,
                    )

                ops2 = ps.tile([64, 2 * NCOL], f32)
                for ab in range(2):
                    for half in range(2):
                        t_ = 2 * half + ab
                        nc.tensor.matmul(
                            out=ops2[0:64, half * NCOL:(half + 1) * NCOL],
                            lhsT=lam_sb[:, 64 * ab:64 * (ab + 1)],
                            rhs=fsb[:, t_ * NCOL:(t_ + 1) * NCOL],
                            start=(ab == 0),
                            stop=(ab == 1),
                            tile_position=(0, 0),
                            skip_group_check=True,
                        )
                osb = sb.tile([128, NCOL], f32)
                nc.vector.tensor_copy(out=osb[0:64, :], in_=ops2[0:64, 0:NCOL])
                nc.vector.tensor_copy(
                    out=osb[64:128, :], in_=ops2[0:64, NCOL:2 * NCOL]
                )

                tps = ps.tile([128, NCOL], f32)
                for blk in range(4):
                    nc.tensor.transpose(
                        out=tps[:, blk * 128:(blk + 1) * 128],
                        in_=osb[:, blk * 128:(blk + 1) * 128],
                        identity=ident[:],
                    )
                oub = sb.tile([128, NCOL], f32)
                nc.vector.tensor_copy(out=oub[:], in_=tps[:])
                nc.sync.dma_start(out=out_r[T], in_=oub[:])

    nc.finalize()
    return nc


def _ensure_ntff_hook():
    """Install the axon NTFF profiling hook if the image's antenv lacks it."""
    import sys
    import types
    try:
        from antenv.axon_hooks import get_axon_ntff_profile_hook  # noqa: F401
        return
    except ImportError:
        pass
    try:
        import antenv
        from trn_agent_boot.trn_boot import _ntff_profile_via_ctypes
        mod = types.ModuleType("antenv.axon_hooks")
        store = [None]
        mod.set_axon_ntff_profile_hook = lambda h: store.__setitem__(0, h)
        mod.get_axon_ntff_profile_hook = lambda: store[0]
        sys.modules["antenv.axon_hooks"] = mod
        antenv.axon_hooks = mod
        mod.set_axon_ntff_profile_hook(
            _ntff_profile_via_ctypes("/opt/axon/libaxon_pjrt.so")
        )
        import concourse.bass_utils as bu
        bu.upload_artifacts = lambda tmpdir: tmpdir
    except Exception as e:  # profiling is best-effort
        print(f"ntff hook install failed: {e}")


def kernel(t, state, cmd, coeffs):
    from concourse.bass_utils import run_bass_kernel_spmd

    trace = bool(int(os.environ.get("BOAT_TRACE", "0")))
    if trace:
        _ensure_ntff_hook()

    t = np.asarray(t)
    state = np.ascontiguousarray(np.asarray(state, dtype=np.float32))
    cmd = np.asarray(cmd, dtype=np.float32)
    coeffs = np.asarray(coeffs, dtype=np.float32)

    precise = bool(int(os.environ.get("BOAT_PRECISE", "0")))
    force_j15 = bool(int(os.environ.get("BOAT_J15", "0")))

    use_j8 = False
    if not force_j15:
        Q = _build_Q(t, cmd, coeffs)
        qscale = max(np.abs(Q).max(), 1e-30)
        resid, amp, Wj8, lamj8 = _solve_j8(Q)
        use_j8 = resid < 1e-7 * qscale
        if not use_j8:
            print(f"J8 fit failed (resid {resid:.2e}, scale {qscale:.2e}); "
                  "falling back to J15 path")

    if use_j8:
        wexp, biasv, lamblk = _host_weights_j8(Wj8, lamj8)
        wexp = wexp.astype(np.float16)
        lamblk = lamblk.astype(np.float16)
        state16 = state.astype(np.float16)
        key = ("j8", precise)
        if key not in _NC_CACHE:
            _NC_CACHE[key] = _build_nc_j8(precise)
        nc = _NC_CACHE[key]
        in_maps = []
        for k in range(NCORES):
            shard = np.ascontiguousarray(state16[k * BS:(k + 1) * BS])
            in_maps.append(
                {"state": shard, "wexp": wexp, "biasv": biasv, "lam": lamblk}
            )
    else:
        wexp4, biasw, lam = _host_weights_j15(t, cmd, coeffs)
        key = ("j15", precise)
        if key not in _NC_CACHE:
            _NC_CACHE[key] = _build_nc_j15(precise)
        nc = _NC_CACHE[key]
        in_maps = []
        for k in range(NCORES):
            shard = np.ascontiguousarray(state[k * BS:(k + 1) * BS])
            in_maps.append(
                {"state": shard, "wexp": wexp4, "biasw": biasw, "lam": lam}
            )

    res = run_bass_kernel_spmd(
        nc,
        in_maps,
        core_ids=list(range(NCORES)),
        trace=trace,
    )
    LAST_RESULT[0] = res
    full = np.concatenate([r["out"] for r in res.results], axis=0)
    if full.dtype != np.float32:
        full = full.astype(np.float32)
    return full


# revision 14
# speedup vs baseline: 1.0754x; 1.0754x over previous
"""Trainium2 Bass kernel for the Boat Dynamic System problem.

Math: out[b, c] = sum_f V[b, f] * coeffs[c, f] where V = [base, pro*base,
rud*base] and base = 15 quadratic monomials of s = (u, v, r, Pf).

Folding pro/rud (scalars picked from cmd on the host) gives an effective
[4, 15] coefficient matrix, i.e. out_c = s~^T Q_c s~ with s~ = (1, u, v, r, Pf).

J8 path (default): only the 4-dim span {Q_c} must be expressed, so a
Levenberg-Marquardt fit on the host finds J=8 vectors w_j in R^5 and
lam [4, 8] with Q_c = sum_j lam_cj w_j w_j^T (72 unknowns >= 60 equations;
exact for generic inputs, residual checked, falls back to the J=15 path).

Device pipeline per [128, 512] tile (16384 batch elements, per core):
  1. contiguous DMA of state (natural layout)
  2. PE transposes of four [128, 128] blocks -> partition q = 4n+f
     (n = batch-sub 0..31, f = component), column = batch-chunk
  3. DVE evac PSUM -> SBUF
  4. M1: two row-tiled K=64 matmuls (strips at tile rows 0/64) project the
     16 records per strip-column onto the 8 w_j -> Y [128, 1024] PSUM
  5. ACT Square(Y + bias) -> fsb [128, 1024] SBUF (bias = w_j0 constant)
  6. M2 fused with the output transpose: for each 128-col block of fsb,
     matmul(lhsT=fsb_block, rhs=lamblk [128, 64]) emits [128, 64] PSUM that
     is already batch-natural (partition = record-group, free = 16*(rec,c))
  7. DVE evac, contiguous DMA out

Matmul dtype is fp32r (full-rate; tf32-like rounding) or fp32 (quarter-rate,
exact) via BOAT_PRECISE=1. BOAT_J15=1 forces the legacy 15-square pipeline.
"""

import os

import numpy as np

NCORES = 8
B = 2097152
BS = B // NCORES          # 262144 rows per core
DT = 0.01
NTILES = 16               # tiles per core
TILE_B = BS // NTILES     # 16384 batch elements per tile
NCOL = 512                # columns per tile (32 batch elements per column)

_PAIRS = [(a, b) for a in range(5) for b in range(a, 5)]  # 15 (a<=b) pairs
_MONO2FEAT = {
    (0, 0): 0, (0, 1): 1, (0, 2): 2, (0, 3): 3, (0, 4): 4,
    (1, 1): 5, (1, 2): 6, (1, 3): 7, (1, 4): 8,
    (2, 2): 9, (2, 3): 10, (2, 4): 11,
    (3, 3): 12, (3, 4): 13,
    (4, 4): 14,
}

_NC_CACHE = {}
LAST_RESULT = [None]


# ---------------------------------------------------------------------------
# host math: effective quadratic forms and the J=8 decomposition
# ---------------------------------------------------------------------------

def _build_Q(t, cmd, coeffs):
    """Q [4, 5, 5] symmetric with out_c = s~^T Q_c s~, s~ = (1, u, v, r, Pf)."""
    idx = int(np.round(float(np.asarray(t).reshape(-1)[0]) / DT))
    pro = float(cmd[idx, 0])
    rud = float(cmd[idx, 1])
    cf = np.asarray(coeffs, dtype=np.float64)
    ceff = cf[:, 0:15] + pro * cf[:, 15:30] + rud * cf[:, 30:45]  # [4, 15]
    Q = np.zeros((4, 5, 5))
    for m, (x, y) in enumerate(_PAIRS):
        g = ceff[:, _MONO2FEAT[(x, y)]]
        if x == y:
            Q[:, x, y] += g
        else:
            Q[:, x, y] += g / 2
            Q[:, y, x] += g / 2
    return Q


def _j8_residual(W, lam, Q):
    M = np.einsum('ja,jb->jab', W, W)
    fit = np.einsum('cj,jab->cab', lam, M)
    R = Q - fit
    iu = np.triu_indices(5)
    return R[:, iu[0], iu[1]].ravel()


def _j8_jac(W, lam):
    J, D = W.shape
    C = lam.shape[0]
    iu = np.triu_indices(D)
    Jm = np.zeros((C * len(iu[0]), J * D + C * J))
    for c in range(C):
        for k, (a, b) in enumerate(zip(*iu)):
            r = c * 15 + k
            for j in range(J):
                Jm[r, J * D + c * J + j] = -W[j, a] * W[j, b]
                Jm[r, j * D + a] += -lam[c, j] * W[j, b]
                Jm[r, j * D + b] += -lam[c, j] * W[j, a]
    return Jm


def _solve_j8(Q, J=8, iters=250):
    """LM with restarts; returns (resid, amp, W [8,5], lam [4,8])."""
    rng = np.random.default_rng(0)
    best = None
    for trial in range(16):
        W = rng.normal(size=(J, 5))
        lam = rng.normal(size=(4, J)) * 0.3
        mu = 1e-3
        for _ in range(iters):
            r = _j8_residual(W, lam, Q)
            f = r @ r
            Jm = _j8_jac(W, lam)
            H = Jm.T @ Jm + mu * np.eye(Jm.shape[1])
            try:
                step = np.linalg.solve(H, Jm.T @ r)
            except np.linalg.LinAlgError:
                break
            Wn = W - step[:J * 5].reshape(J, 5)
            ln = lam - step[J * 5:].reshape(4, J)
            rn = _j8_residual(Wn, ln, Q)
            if rn @ rn < f:
                W, lam = Wn, ln
                mu = max(mu * 0.5, 1e-12)
                if rn @ rn < 1e-24:
                    break
            else:
                mu *= 4.0
                if mu > 1e12:
                    break
        r = _j8_residual(W, lam, Q)
        f = float(np.sqrt(r @ r))
        s = np.linalg.norm(W, axis=1, keepdims=True)
        s[s == 0] = 1
        W2 = W / s
        lam2 = lam * (s.ravel() ** 2)[None, :]
        amp = float(np.abs(lam2).sum())
        if best is None or (f, amp) < (best[0], best[1]):
            best = (f, amp, W2, lam2)
        if f < 1e-10 and amp < 60:
            break
    return best


def _host_weights_j8(W, lam):
    """wexp [128, 128], biasv [128, 1], lamblk [128, 64] device constants."""
    wexp = np.zeros((128, 128), dtype=np.float32)
    for s in range(2):
        for v in range(16):
            for f in range(4):
                for j in range(8):
                    wexp[64 * s + 4 * v + f, 8 * v + j] = W[j, 1 + f]
    biasv = np.zeros((128, 1), dtype=np.float32)
    for v in range(16):
        for j in range(8):
            biasv[8 * v + j, 0] = W[j, 0]
    lamblk = np.zeros((128, 64), dtype=np.float32)
    for v in range(16):
        for j in range(8):
            for c in range(4):
                lamblk[8 * v + j, 4 * v + c] = lam[c, j]
    return wexp, biasv, lamblk


def _build_nc_j8(precise: bool):
    """fp16 pipeline: DMA-xbar input transpose, fp16 matmuls, fp16 output.

    fp16 inputs carry a 10-bit mantissa -- the same effective precision as
    fp32r matmuls -- but run at full PE rate with fast weight load, and
    2-byte dtypes unlock the DMA transpose engine (input lands f-major in
    SBUF with no PE/DVE work) and halve both DMA directions.
    """
    import concourse.bacc as bacc
    import concourse.mybir as mybir
    import concourse.tile as tile

    nc = bacc.Bacc("TRN2", target_bir_lowering=False, debug=False)
    f32 = mybir.dt.float32
    f16 = mybir.dt.float16
    Square = mybir.ActivationFunctionType.Square

    state = nc.dram_tensor("state", [BS, 4], f16, kind="ExternalInput")
    wexp_d = nc.dram_tensor("wexp", [128, 128], f16, kind="ExternalInput")
    biasv_d = nc.dram_tensor("biasv", [128, 1], f32, kind="ExternalInput")
    lam_d = nc.dram_tensor("lam", [128, 64], f16, kind="ExternalInput")
    out = nc.dram_tensor("out", [BS, 4], f16, kind="ExternalOutput")

    NMEGA = 4                  # DMA granularity: 4 compute-tiles per transfer
    MT = NTILES // NMEGA       # 4 compute-tiles per mega
    MCOL = MT * NCOL           # 2048 ssb columns per mega

    # per mega: X [2048, 128] with row c2 = one record-group of 32 records
    # (128 contiguous fp16); the xbar transpose lands it f-major in SBUF.
    state_r = state[:, :].rearrange(
        "(M c2 n) f -> M c2 (n f)", M=NMEGA, c2=MCOL, n=32
    )
    # stride-4 M2 slicing puts 128 consecutive records on each partition:
    # osb[p, 4*n + c] = out[M*65536 + T*16384 + 128*p + n, c]  (1KB chunks)
    out_r = out[:, :].rearrange(
        "(M T p n) f -> M p T n f", M=NMEGA, T=MT, p=128, n=128
    )

    with tile.TileContext(nc) as tc:
        with (
            tc.tile_pool(name="consts", bufs=1) as cpool,
            tc.tile_pool(name="si", bufs=3) as si,
            tc.tile_pool(name="so", bufs=2) as so,
            tc.tile_pool(name="fs", bufs=6) as fs,
            tc.tile_pool(name="ps", bufs=2, space="PSUM") as ps,
            tc.tile_pool(name="po", bufs=3, space="PSUM") as po,
        ):
            # PE warmup: dependency-free matmuls run during the DMA prefetch
            # window so HAM un-throttles the PE clock before real work lands
            wdum = cpool.tile([128, 128], f16)
            nc.gpsimd.memset(wdum[:], 0.0)
            rdum = cpool.tile([128, NCOL], f16)
            nc.gpsimd.memset(rdum[:], 0.0)
            pdum = ps.tile([128, NCOL], f32, tag="warm", bufs=1)
            for i in range(8):
                nc.tensor.matmul(
                    out=pdum[:],
                    lhsT=wdum[:],
                    rhs=rdum[:],
                    start=True,
                    stop=True,
                    tile_position=(0, 0),
                    skip_group_check=True,
                )

            # input transposes first in program order: the sync HWDGE queue
            # starts streaming them immediately; consts go on the scalar queue
            ssb_t = []
            for M in range(NMEGA):
                # xbar: ssb[4n+f, c2] = state[M*65536 + 32*c2 + n, f]
                ssb = si.tile([128, MCOL], f16, name=f"ssb{M}", tag="ssb")
                if M == 0:
                    # small lead chunk so the first compute tile starts early
                    nc.sync.dma_start(
                        out=ssb[:, 0:NCOL], in_=state_r[0, 0:NCOL],
                        transpose=True,
                    )
                    nc.sync.dma_start(
                        out=ssb[:, NCOL:MCOL], in_=state_r[0, NCOL:MCOL],
                        transpose=True,
                    )
                else:
                    nc.sync.dma_start(
                        out=ssb[:], in_=state_r[M], transpose=True
                    )
                ssb_t.append(ssb)

            # consts via SWDGE (gpsimd): separate descriptor rings, so they
            # don't trip the HWDGE transpose serialization guard
            wexp_sb = cpool.tile([128, 128], f16)
            nc.gpsimd.dma_start(out=wexp_sb[:], in_=wexp_d[:, :])
            biasv_sb = cpool.tile([128, 1], f32)
            nc.gpsimd.dma_start(out=biasv_sb[:], in_=biasv_d[:, :])
            lam_sb = cpool.tile([128, 64], f16)
            nc.gpsimd.dma_start(out=lam_sb[:], in_=lam_d[:, :])

            for M in range(NMEGA):
                ssb = ssb_t[M]
                osb = so.tile([128, MCOL], f16)

                for c in range(MT):
                    sl = slice(c * NCOL, (c + 1) * NCOL)
                    # M1: 2 row-tiled K=64 strips -> Y [128, 1024] (2 banks)
                    yps = ps.tile([128, 2 * NCOL], f32)
                    for s in range(2):
                        nc.tensor.matmul(
                            out=yps[:, s * NCOL:(s + 1) * NCOL],
                            lhsT=wexp_sb[64 * s:64 * (s + 1), :],
                            rhs=ssb[64 * s:64 * (s + 1), sl],
                            start=True,
                            stop=True,
                            tile_position=(64 * s, 0),
                        )

                    # squares (bias folds the w_j0 constant component)
                    fsb = fs.tile([128, 2 * NCOL], f16)
                    nc.scalar.activation(
                        out=fsb[:],
                        in_=yps[:],
                        func=Square,
                        bias=biasv_sb[:, 0:1],
                        scale=1.0,
                    )

                    # M2 fused with output transpose: lhsT = squared
                    # features (stride-4 cols: partition m = group 4m+phi),
                    # rhs = block-diag lambda; out batch-natural, 1KB chunks.
                    ops = po.tile([128, NCOL], f32)
                    for s in range(2):
                        for phi in range(4):
                            c0 = 128 * phi + 64 * s
                            nc.tensor.matmul(
                                out=ops[:, c0:c0 + 64],
                                lhsT=fsb[:, NCOL * s + phi:NCOL * (s + 1):4],
                                rhs=lam_sb[:],
                                start=True,
                                stop=True,
                                tile_position=(0, 0),
                            )
                    nc.vector.tensor_copy(out=osb[:, sl], in_=ops[:])
                # output via SWDGE: keeps the HWDGE ring free for the input
                # transposes (which serialize against any in-flight HWDGE DMA)
                nc.gpsimd.dma_start(out=out_r[M], in_=osb[:])

    nc.finalize()
    return nc


# ---------------------------------------------------------------------------
# legacy J=15 pipeline (fallback when the J=8 fit fails)
# ---------------------------------------------------------------------------

def _square_basis():
    """15 fixed vectors w_j in R^5 whose squared functionals span quadratics."""
    W5 = np.zeros((15, 5), dtype=np.float64)
    for j, (a, b) in enumerate(_PAIRS):
        W5[j, a] += 1.0
        if b != a:
            W5[j, b] += 1.0
    M = np.zeros((15, 15), dtype=np.float64)
    for m, (x, y) in enumerate(_PAIRS):
        for j in range(15):
            M[m, j] = W5[j, x] * W5[j, y] * (1.0 if x == y else 2.0)
    return W5, M


def _host_weights_j15(t, cmd, coeffs):
    """Fold cmd/coeffs into the device weight tensors (all tiny)."""
    idx = int(np.round(float(np.asarray(t).reshape(-1)[0]) / DT))
    pro = float(cmd[idx, 0])
    rud = float(cmd[idx, 1])
    cf = np.asarray(coeffs, dtype=np.float64)
    ceff = cf[:, 0:15] + pro * cf[:, 15:30] + rud * cf[:, 30:45]  # [4, 15]

    gamma = np.zeros((4, 15), dtype=np.float64)
    for m, (x, y) in enumerate(_PAIRS):
        gamma[:, m] = ceff[:, _MONO2FEAT[(x, y)]]

    W5, M = _square_basis()
    lam45 = np.linalg.solve(M, gamma.T).T  # [4, 15]

    wexp4 = np.zeros((128, 120), dtype=np.float32)
    for t_ in range(4):
        for g in range(8):
            for j in range(15):
                for f in range(4):
                    wexp4[32 * t_ + 4 * g + f, g * 15 + j] = W5[j, 1 + f]

    biasw = np.zeros((120, 1), dtype=np.float32)
    for g in range(8):
        for j in range(15):
            biasw[g * 15 + j, 0] = W5[j, 0]

    lam = np.zeros((120, 32), dtype=np.float32)
    for g in range(8):
        for j in range(15):
            for c in range(4):
                lam[g * 15 + j, 4 * g + c] = lam45[c, j]

    lamAB = np.zeros((120, 128), dtype=np.float32)
    lamAB[:, 0:32] = lam
    lamAB[:, 96:128] = lam

    return wexp4, biasw, lamAB


def _build_nc_j15(precise: bool):
    import concourse.bacc as bacc
    import concourse.mybir as mybir
    import concourse.tile as tile
    from concourse.masks import make_identity

    nc = bacc.Bacc("TRN2", target_bir_lowering=False, debug=False)
    f32 = mybir.dt.float32
    mmdt = f32 if precise else mybir.dt.float32r
    Square = mybir.ActivationFunctionType.Square

    state = nc.dram_tensor("state", [BS, 4], mmdt, kind="ExternalInput")
    wexp_d = nc.dram_tensor("wexp", [128, 120], mmdt, kind="ExternalInput")
    biasw_d = nc.dram_tensor("biasw", [120, 1], f32, kind="ExternalInput")
    lam_d = nc.dram_tensor("lam", [120, 128], mmdt, kind="ExternalInput")
    out = nc.dram_tensor("out", [BS, 4], f32, kind="ExternalOutput")

    state_r = state[:, :].rearrange(
        "(T blk p n) f -> T p blk n f", T=NTILES, blk=4, p=128, n=32
    )
    out_r = out[:, :].rearrange(
        "(T blk p n) f -> T p blk n f", T=NTILES, blk=4, p=128, n=32
    )

    with tile.TileContext(nc) as tc:
        with (
            tc.tile_pool(name="consts", bufs=1) as cpool,
            tc.tile_pool(name="sb", bufs=4) as sb,
            tc.tile_pool(name="ps", bufs=1, space="PSUM") as ps,
        ):
            ident = cpool.tile([128, 128], f32)
            make_identity(nc, ident[:])
            identr = cpool.tile([128, 128], mmdt)
            nc.vector.tensor_copy(out=identr[:], in_=ident[:])
            wexp_sb = cpool.tile([128, 120], mmdt)
            nc.sync.dma_start(out=wexp_sb[:], in_=wexp_d[:, :])
            biasw_sb = cpool.tile([120, 1], f32)
            nc.sync.dma_start(out=biasw_sb[:], in_=biasw_d[:, :])
            lam_sb = cpool.tile([120, 128], mmdt)
            nc.sync.dma_start(out=lam_sb[:], in_=lam_d[:, :])

            for T in range(NTILES):
                xn = sb.tile([128, NCOL], mmdt)
                nc.sync.dma_start(out=xn[:], in_=state_r[T])

                spsum = ps.tile([128, NCOL], mmdt)
                for blk in range(4):
                    nc.tensor.transpose(
                        out=spsum[:, blk * 128:(blk + 1) * 128],
                        in_=xn[:, blk * 128:(blk + 1) * 128],
                        identity=identr[:],
                    )
                ssb = sb.tile([128, NCOL], mmdt)
                nc.vector.tensor_copy(out=ssb[:], in_=spsum[:])

                fsb = sb.tile([120, 4 * NCOL], mmdt)
                for h in range(2):
                    yps = ps.tile([120, 2 * NCOL], f32, tag=f"y{h}")
                    for u in range(2):
                        t_ = 2 * h + u
                        nc.tensor.matmul(
                            out=yps[:, u * NCOL:(u + 1) * NCOL],
                            lhsT=wexp_sb[32 * t_:32 * (t_ + 1), :],
                            rhs=ssb[32 * t_:32 * (t_ + 1), :],
                            start=True,
                            stop=True,
                            tile_position=(32 * t_, 0),
                        )
                    nc.scalar.activation(
                        out=fsb[:, h * 2 * NCOL:(h + 1) * 2 * NCOL],
                        in_=yps[:],
                        func=Square,
                        bias=biasw_sb[:, 0:1],
                        scale=1.0,
                    )

                ops2 = ps.tile([64, 2 * NCOL], f32)
                for ab in range(2):
                    for half in range(2):
                        t_ = 2 * half + ab
                        nc.tensor.matmul(
                            out=ops2[0:64, half * NCOL:(half + 1) * NCOL],
                            lhsT=lam_sb[:, 64 * ab:64 * (ab + 1)],
                            rhs=fsb[:, t_ * NCOL:(t_ + 1) * NCOL],
                            start=(ab == 0),
                            stop=(ab == 1),
                            tile_position=(0, 0),
                            skip_group_check=True,
                        )
                osb = sb.tile([128, NCOL], f32)
                nc.vector.tensor_copy(out=osb[0:64, :], in_=ops2[0:64, 0:NCOL])
                nc.vector.tensor_copy(
                    out=osb[64:128, :], in_=ops2[0:64, NCOL:2 * NCOL]
                )

                tps = ps.tile([128, NCOL], f32)
                for blk in range(4):
                    nc.tensor.transpose(
                        out=tps[:, blk * 128:(blk + 1) * 128],
                        in_=osb[:, blk * 128:(blk + 1) * 128],
                        identity=ident[:],
                    )
                oub = sb.tile([128, NCOL], f32)
                nc.vector.tensor_copy(out=oub[:], in_=tps[:])
                nc.sync.dma_start(out=out_r[T], in_=oub[:])

    nc.finalize()
    return nc


def _ensure_ntff_hook():
    """Install the axon NTFF profiling hook if the image's antenv lacks it."""
    import sys
    import types
    try:
        from antenv.axon_hooks import get_axon_ntff_profile_hook  # noqa: F401
        return
    except ImportError:
        pass
    try:
        import antenv
        from trn_agent_boot.trn_boot import _ntff_profile_via_ctypes
        mod = types.ModuleType("antenv.axon_hooks")
        store = [None]
        mod.set_axon_ntff_profile_hook = lambda h: store.__setitem__(0, h)
        mod.get_axon_ntff_profile_hook = lambda: store[0]
        sys.modules["antenv.axon_hooks"] = mod
        antenv.axon_hooks = mod
        mod.set_axon_ntff_profile_hook(
            _ntff_profile_via_ctypes("/opt/axon/libaxon_pjrt.so")
        )
        import concourse.bass_utils as bu
        bu.upload_artifacts = lambda tmpdir: tmpdir
    except Exception as e:  # profiling is best-effort
        print(f"ntff hook install failed: {e}")


def kernel(t, state, cmd, coeffs):
    from concourse.bass_utils import run_bass_kernel_spmd

    trace = bool(int(os.environ.get("BOAT_TRACE", "0")))
    if trace:
        _ensure_ntff_hook()

    t = np.asarray(t)
    state = np.ascontiguousarray(np.asarray(state, dtype=np.float32))
    cmd = np.asarray(cmd, dtype=np.float32)
    coeffs = np.asarray(coeffs, dtype=np.float32)

    precise = bool(int(os.environ.get("BOAT_PRECISE", "0")))
    force_j15 = bool(int(os.environ.get("BOAT_J15", "0")))

    use_j8 = False
    if not force_j15:
        Q = _build_Q(t, cmd, coeffs)
        qscale = max(np.abs(Q).max(), 1e-30)
        resid, amp, Wj8, lamj8 = _solve_j8(Q)
        use_j8 = resid < 1e-7 * qscale
        if not use_j8:
            print(f"J8 fit failed (resid {resid:.2e}, scale {qscale:.2e}); "
                  "falling back to J15 path")

    if use_j8:
        wexp, biasv, lamblk = _host_weights_j8(Wj8, lamj8)
        wexp = wexp.astype(np.float16)
        lamblk = lamblk.astype(np.float16)
        state16 = state.astype(np.float16)
        key = ("j8", precise)
        if key not in _NC_CACHE:
            _NC_CACHE[key] = _build_nc_j8(precise)
        nc = _NC_CACHE[key]
        in_maps = []
        for k in range(NCORES):
            shard = np.ascontiguousarray(state16[k * BS:(k + 1) * BS])
            in_maps.append(
                {"state": shard, "wexp": wexp, "biasv": biasv, "lam": lamblk}
            )
    else:
        wexp4, biasw, lam = _host_weights_j15(t, cmd, coeffs)
        key = ("j15", precise)
        if key not in _NC_CACHE:
            _NC_CACHE[key] = _build_nc_j15(precise)
        nc = _NC_CACHE[key]
        in_maps = []
        for k in range(NCORES):
            shard = np.ascontiguousarray(state[k * BS:(k + 1) * BS])
            in_maps.append(
                {"state": shard, "wexp": wexp4, "biasw": biasw, "lam": lam}
            )

    res = run_bass_kernel_spmd(
        nc,
        in_maps,
        core_ids=list(range(NCORES)),
        trace=trace,
    )
    LAST_RESULT[0] = res
    full = np.concatenate([r["out"] for r in res.results], axis=0)
    if full.dtype != np.float32:
        full = full.astype(np.float32)
    return full


# revision 15
# speedup vs baseline: 1.1866x; 1.1034x over previous
"""Trainium2 Bass kernel for the Boat Dynamic System problem.

Math: out[b, c] = sum_f V[b, f] * coeffs[c, f] where V = [base, pro*base,
rud*base] and base = 15 quadratic monomials of s = (u, v, r, Pf).

Folding pro/rud (scalars picked from cmd on the host) gives an effective
[4, 15] coefficient matrix, i.e. out_c = s~^T Q_c s~ with s~ = (1, u, v, r, Pf).

J8 path (default): only the 4-dim span {Q_c} must be expressed, so a
Levenberg-Marquardt fit on the host finds J=8 vectors w_j in R^5 and
lam [4, 8] with Q_c = sum_j lam_cj w_j w_j^T (72 unknowns >= 60 equations;
exact for generic inputs, residual checked, falls back to the J=15 path).

Device pipeline per [128, 512] tile (16384 batch elements, per core):
  1. contiguous DMA of state (natural layout)
  2. PE transposes of four [128, 128] blocks -> partition q = 4n+f
     (n = batch-sub 0..31, f = component), column = batch-chunk
  3. DVE evac PSUM -> SBUF
  4. M1: two row-tiled K=64 matmuls (strips at tile rows 0/64) project the
     16 records per strip-column onto the 8 w_j -> Y [128, 1024] PSUM
  5. ACT Square(Y + bias) -> fsb [128, 1024] SBUF (bias = w_j0 constant)
  6. M2 fused with the output transpose: for each 128-col block of fsb,
     matmul(lhsT=fsb_block, rhs=lamblk [128, 64]) emits [128, 64] PSUM that
     is already batch-natural (partition = record-group, free = 16*(rec,c))
  7. DVE evac, contiguous DMA out

Matmul dtype is fp32r (full-rate; tf32-like rounding) or fp32 (quarter-rate,
exact) via BOAT_PRECISE=1. BOAT_J15=1 forces the legacy 15-square pipeline.
"""

import os

import numpy as np

NCORES = 8
B = 2097152
BS = B // NCORES          # 262144 rows per core
DT = 0.01
NTILES = 16               # tiles per core
TILE_B = BS // NTILES     # 16384 batch elements per tile
NCOL = 512                # columns per tile (32 batch elements per column)

_PAIRS = [(a, b) for a in range(5) for b in range(a, 5)]  # 15 (a<=b) pairs
_MONO2FEAT = {
    (0, 0): 0, (0, 1): 1, (0, 2): 2, (0, 3): 3, (0, 4): 4,
    (1, 1): 5, (1, 2): 6, (1, 3): 7, (1, 4): 8,
    (2, 2): 9, (2, 3): 10, (2, 4): 11,
    (3, 3): 12, (3, 4): 13,
    (4, 4): 14,
}

_NC_CACHE = {}
LAST_RESULT = [None]


# ---------------------------------------------------------------------------
# host math: effective quadratic forms and the J=8 decomposition
# ---------------------------------------------------------------------------

def _build_Q(t, cmd, coeffs):
    """Q [4, 5, 5] symmetric with out_c = s~^T Q_c s~, s~ = (1, u, v, r, Pf)."""
    idx = int(np.round(float(np.asarray(t).reshape(-1)[0]) / DT))
    pro = float(cmd[idx, 0])
    rud = float(cmd[idx, 1])
    cf = np.asarray(coeffs, dtype=np.float64)
    ceff = cf[:, 0:15] + pro * cf[:, 15:30] + rud * cf[:, 30:45]  # [4, 15]
    Q = np.zeros((4, 5, 5))
    for m, (x, y) in enumerate(_PAIRS):
        g = ceff[:, _MONO2FEAT[(x, y)]]
        if x == y:
            Q[:, x, y] += g
        else:
            Q[:, x, y] += g / 2
            Q[:, y, x] += g / 2
    return Q


def _j8_residual(W, lam, Q):
    M = np.einsum('ja,jb->jab', W, W)
    fit = np.einsum('cj,jab->cab', lam, M)
    R = Q - fit
    iu = np.triu_indices(5)
    return R[:, iu[0], iu[1]].ravel()


def _j8_jac(W, lam):
    J, D = W.shape
    C = lam.shape[0]
    iu = np.triu_indices(D)
    Jm = np.zeros((C * len(iu[0]), J * D + C * J))
    for c in range(C):
        for k, (a, b) in enumerate(zip(*iu)):
            r = c * 15 + k
            for j in range(J):
                Jm[r, J * D + c * J + j] = -W[j, a] * W[j, b]
                Jm[r, j * D + a] += -lam[c, j] * W[j, b]
                Jm[r, j * D + b] += -lam[c, j] * W[j, a]
    return Jm


def _solve_j8(Q, J=8, iters=250):
    """LM with restarts; returns (resid, amp, W [8,5], lam [4,8])."""
    rng = np.random.default_rng(0)
    best = None
    for trial in range(16):
        W = rng.normal(size=(J, 5))
        lam = rng.normal(size=(4, J)) * 0.3
        mu = 1e-3
        for _ in range(iters):
            r = _j8_residual(W, lam, Q)
            f = r @ r
            Jm = _j8_jac(W, lam)
            H = Jm.T @ Jm + mu * np.eye(Jm.shape[1])
            try:
                step = np.linalg.solve(H, Jm.T @ r)
            except np.linalg.LinAlgError:
                break
            Wn = W - step[:J * 5].reshape(J, 5)
            ln = lam - step[J * 5:].reshape(4, J)
            rn = _j8_residual(Wn, ln, Q)
            if rn @ rn < f:
                W, lam = Wn, ln
                mu = max(mu * 0.5, 1e-12)
                if rn @ rn < 1e-24:
                    break
            else:
                mu *= 4.0
                if mu > 1e12:
                    break
        r = _j8_residual(W, lam, Q)
        f = float(np.sqrt(r @ r))
        s = np.linalg.norm(W, axis=1, keepdims=True)
        s[s == 0] = 1
        W2 = W / s
        lam2 = lam * (s.ravel() ** 2)[None, :]
        amp = float(np.abs(lam2).sum())
        if best is None or (f, amp) < (best[0], best[1]):
            best = (f, amp, W2, lam2)
        if f < 1e-10 and amp < 60:
            break
    return best


def _host_weights_j8(W, lam):
    """wexp [128, 128], biasv [128, 1], lamblk [128, 64] device constants."""
    wexp = np.zeros((128, 128), dtype=np.float32)
    for s in range(2):
        for v in range(16):
            for f in range(4):
                for j in range(8):
                    wexp[64 * s + 4 * v + f, 8 * v + j] = W[j, 1 + f]
    biasv = np.zeros((128, 1), dtype=np.float32)
    for v in range(16):
        for j in range(8):
            biasv[8 * v + j, 0] = W[j, 0]
    lamblk = np.zeros((128, 64), dtype=np.float32)
    for v in range(16):
        for j in range(8):
            for c in range(4):
                lamblk[8 * v + j, 4 * v + c] = lam[c, j]
    return wexp, biasv, lamblk


def _build_nc_j8(precise: bool):
    """fp16 pipeline v10: plain 1KB-chunk DMAs, PE fp16 transposes, fused M2.

    fp16 carries a 10-bit mantissa -- same effective precision as fp32r
    matmuls -- but runs at full PE rate and halves both DMA directions.
    The DMA xbar transpose is avoided entirely: it serializes against every
    other DMA in flight (deadlock guard) and only sustains ~134 GB/s.

    Per compute tile B (16384 records, [128, 512] natural cols = 4n+f):
      transpose 4 blocks on PE -> ssb[4nh+f, 128h+p] (records 128p+32h+nh)
      M1: 2 row-tiled K=64 strips -> Y [128, 1024]; ACT Square(+bias) -> fsb
      M2 fused with the output transpose: lhsT = fsb 128-col block (s, phi),
      rhs = block-diag lambda -> ops[m, 4nu+c] is batch-natural with 128
      consecutive records per partition (1KB contiguous output chunks).
    """
    import concourse.bacc as bacc
    import concourse.mybir as mybir
    import concourse.tile as tile
    from concourse.masks import make_identity

    nc = bacc.Bacc("TRN2", target_bir_lowering=False, debug=False)
    f32 = mybir.dt.float32
    f16 = mybir.dt.float16
    Square = mybir.ActivationFunctionType.Square

    state = nc.dram_tensor("state", [BS, 4], f16, kind="ExternalInput")
    wexp_d = nc.dram_tensor("wexp", [128, 128], f16, kind="ExternalInput")
    biasv_d = nc.dram_tensor("biasv", [128, 1], f32, kind="ExternalInput")
    lam_d = nc.dram_tensor("lam", [128, 64], f16, kind="ExternalInput")
    out = nc.dram_tensor("out", [BS, 4], f16, kind="ExternalOutput")

    NMEGA = 4                  # input DMA granularity: 4 tiles per transfer
    MT = NTILES // NMEGA       # 4 compute-tiles per mega
    MCOL = MT * NCOL           # 2048 xn columns per mega

    # both sides 1KB-contiguous per partition line
    state_r = state[:, :].rearrange(
        "(M B p n) f -> M p B n f", M=NMEGA, B=MT, p=128, n=128
    )
    out_r = out[:, :].rearrange(
        "(T p n) f -> T p n f", T=NTILES, p=128, n=128
    )

    with tile.TileContext(nc) as tc:
        with (
            tc.tile_pool(name="consts", bufs=1) as cpool,
            tc.tile_pool(name="si", bufs=2) as si,
            tc.tile_pool(name="sb", bufs=4) as sb,
            tc.tile_pool(name="so", bufs=3) as so,
            tc.tile_pool(name="fs", bufs=6) as fs,
            tc.tile_pool(name="pt", bufs=2, space="PSUM") as pt,
            tc.tile_pool(name="ps", bufs=2, space="PSUM") as ps,
            tc.tile_pool(name="po", bufs=2, space="PSUM") as po,
        ):
            # PE warmup: dependency-light matmuls during the DMA prefetch
            # window un-throttle the PE clock (HAM) before real work lands
            wdum = cpool.tile([128, 128], f16)
            nc.gpsimd.memset(wdum[:], 0.0)
            rdum = cpool.tile([128, NCOL], f16)
            nc.gpsimd.memset(rdum[:], 0.0)

            # input megas on the sync HWDGE queue, prefetch-first
            xn_t = []
            for M in range(NMEGA):
                xn = si.tile([128, MCOL], f16, name=f"xn{M}", tag="xn")
                nc.sync.dma_start(out=xn[:], in_=state_r[M])
                xn_t.append(xn)

            # consts on the scalar HWDGE queue (idle until first ACTIVATE)
            wexp_sb = cpool.tile([128, 128], f16)
            nc.scalar.dma_start(out=wexp_sb[:], in_=wexp_d[:, :])
            biasv_sb = cpool.tile([128, 1], f32)
            nc.scalar.dma_start(out=biasv_sb[:], in_=biasv_d[:, :])
            lam_sb = cpool.tile([128, 64], f16)
            nc.scalar.dma_start(out=lam_sb[:], in_=lam_d[:, :])

            ident = cpool.tile([128, 128], f32)
            make_identity(nc, ident[:])
            identr = cpool.tile([128, 128], f16)
            nc.vector.tensor_copy(out=identr[:], in_=ident[:])

            for i in range(8):
                wps = ps.tile([128, 2 * NCOL], f32, name="yps", tag="y")
                nc.tensor.matmul(
                    out=wps[:, 0:NCOL],
                    lhsT=wdum[:],
                    rhs=rdum[:],
                    start=True,
                    stop=True,
                    tile_position=(0, 0),
                    skip_group_check=True,
                )

            for T in range(NTILES):
                M, B = T // MT, T % MT
                xn = xn_t[M]

                # PE transpose: ssb[4nh+f, 128h+p], records 128p + 32h + nh
                spsum = pt.tile([128, NCOL], f16)
                for h in range(4):
                    nc.tensor.transpose(
                        out=spsum[:, h * 128:(h + 1) * 128],
                        in_=xn[:, B * NCOL + h * 128:B * NCOL + (h + 1) * 128],
                        identity=identr[:],
                    )
                ssb = sb.tile([128, NCOL], f16)
                nc.vector.tensor_copy(out=ssb[:], in_=spsum[:])

                # M1: 2 row-tiled K=64 strips -> Y [128, 1024] (2 banks)
                yps = ps.tile([128, 2 * NCOL], f32, name="yps", tag="y")
                for s in range(2):
                    nc.tensor.matmul(
                        out=yps[:, s * NCOL:(s + 1) * NCOL],
                        lhsT=wexp_sb[64 * s:64 * (s + 1), :],
                        rhs=ssb[64 * s:64 * (s + 1), :],
                        start=True,
                        stop=True,
                        tile_position=(64 * s, 0),
                    )

                # squares (bias folds the w_j0 constant component)
                fsb = fs.tile([128, 2 * NCOL], f16)
                nc.scalar.activation(
                    out=fsb[:],
                    in_=yps[:],
                    func=Square,
                    bias=biasv_sb[:, 0:1],
                    scale=1.0,
                )

                # M2 fused with output transpose: contiguous 128-col blocks;
                # ops[m, 4nu+c] = record 128m + 32phi + 16s + nu -> natural
                ops = po.tile([128, NCOL], f32)
                for s in range(2):
                    for phi in range(4):
                        c0 = 128 * phi + 64 * s
                        nc.tensor.matmul(
                            out=ops[:, c0:c0 + 64],
                            lhsT=fsb[:, NCOL * s + 128 * phi:
                                     NCOL * s + 128 * (phi + 1)],
                            rhs=lam_sb[:],
                            start=True,
                            stop=True,
                            tile_position=(0, 0),
                        )
                osb = so.tile([128, NCOL], f16)
                nc.vector.tensor_copy(out=osb[:], in_=ops[:])
                # output via SWDGE (gpsimd): off the sync ring, overlaps input
                nc.gpsimd.dma_start(out=out_r[T], in_=osb[:])

    nc.finalize()
    return nc


# ---------------------------------------------------------------------------
# legacy J=15 pipeline (fallback when the J=8 fit fails)
# ---------------------------------------------------------------------------

def _square_basis():
    """15 fixed vectors w_j in R^5 whose squared functionals span quadratics."""
    W5 = np.zeros((15, 5), dtype=np.float64)
    for j, (a, b) in enumerate(_PAIRS):
        W5[j, a] += 1.0
        if b != a:
            W5[j, b] += 1.0
    M = np.zeros((15, 15), dtype=np.float64)
    for m, (x, y) in enumerate(_PAIRS):
        for j in range(15):
            M[m, j] = W5[j, x] * W5[j, y] * (1.0 if x == y else 2.0)
    return W5, M


def _host_weights_j15(t, cmd, coeffs):
    """Fold cmd/coeffs into the device weight tensors (all tiny)."""
    idx = int(np.round(float(np.asarray(t).reshape(-1)[0]) / DT))
    pro = float(cmd[idx, 0])
    rud = float(cmd[idx, 1])
    cf = np.asarray(coeffs, dtype=np.float64)
    ceff = cf[:, 0:15] + pro * cf[:, 15:30] + rud * cf[:, 30:45]  # [4, 15]

    gamma = np.zeros((4, 15), dtype=np.float64)
    for m, (x, y) in enumerate(_PAIRS):
        gamma[:, m] = ceff[:, _MONO2FEAT[(x, y)]]

    W5, M = _square_basis()
    lam45 = np.linalg.solve(M, gamma.T).T  # [4, 15]

    wexp4 = np.zeros((128, 120), dtype=np.float32)
    for t_ in range(4):
        for g in range(8):
            for j in range(15):
                for f in range(4):
                    wexp4[32 * t_ + 4 * g + f, g * 15 + j] = W5[j, 1 + f]

    biasw = np.zeros((120, 1), dtype=np.float32)
    for g in range(8):
        for j in range(15):
            biasw[g * 15 + j, 0] = W5[j, 0]

    lam = np.zeros((120, 32), dtype=np.float32)
    for g in range(8):
        for j in range(15):
            for c in range(4):
                lam[g * 15 + j, 4 * g + c] = lam45[c, j]

    lamAB = np.zeros((120, 128), dtype=np.float32)
    lamAB[:, 0:32] = lam
    lamAB[:, 96:128] = lam

    return wexp4, biasw, lamAB


def _build_nc_j15(precise: bool):
    import concourse.bacc as bacc
    import concourse.mybir as mybir
    import concourse.tile as tile
    from concourse.masks import make_identity

    nc = bacc.Bacc("TRN2", target_bir_lowering=False, debug=False)
    f32 = mybir.dt.float32
    mmdt = f32 if precise else mybir.dt.float32r
    Square = mybir.ActivationFunctionType.Square

    state = nc.dram_tensor("state", [BS, 4], mmdt, kind="ExternalInput")
    wexp_d = nc.dram_tensor("wexp", [128, 120], mmdt, kind="ExternalInput")
    biasw_d = nc.dram_tensor("biasw", [120, 1], f32, kind="ExternalInput")
    lam_d = nc.dram_tensor("lam", [120, 128], mmdt, kind="ExternalInput")
    out = nc.dram_tensor("out", [BS, 4], f32, kind="ExternalOutput")

    state_r = state[:, :].rearrange(
        "(T blk p n) f -> T p blk n f", T=NTILES, blk=4, p=128, n=32
    )
    out_r = out[:, :].rearrange(
        "(T blk p n) f -> T p blk n f", T=NTILES, blk=4, p=128, n=32
    )

    with tile.TileContext(nc) as tc:
        with (
            tc.tile_pool(name="consts", bufs=1) as cpool,
            tc.tile_pool(name="sb", bufs=4) as sb,
            tc.tile_pool(name="ps", bufs=1, space="PSUM") as ps,
        ):
            ident = cpool.tile([128, 128], f32)
            make_identity(nc, ident[:])
            identr = cpool.tile([128, 128], mmdt)
            nc.vector.tensor_copy(out=identr[:], in_=ident[:])
            wexp_sb = cpool.tile([128, 120], mmdt)
            nc.sync.dma_start(out=wexp_sb[:], in_=wexp_d[:, :])
            biasw_sb = cpool.tile([120, 1], f32)
            nc.sync.dma_start(out=biasw_sb[:], in_=biasw_d[:, :])
            lam_sb = cpool.tile([120, 128], mmdt)
            nc.sync.dma_start(out=lam_sb[:], in_=lam_d[:, :])

            for T in range(NTILES):
                xn = sb.tile([128, NCOL], mmdt)
                nc.sync.dma_start(out=xn[:], in_=state_r[T])

                spsum = ps.tile([128, NCOL], mmdt)
                for blk in range(4):
                    nc.tensor.transpose(
                        out=spsum[:, blk * 128:(blk + 1) * 128],
                        in_=xn[:, blk * 128:(blk + 1) * 128],
                        identity=identr[:],
                    )
                ssb = sb.tile([128, NCOL], mmdt)
                nc.vector.tensor_copy(out=ssb[:], in_=spsum[:])

                fsb = sb.tile([120, 4 * NCOL], mmdt)
                for h in range(2):
                    yps = ps.tile([120, 2 * NCOL], f32, tag=f"y{h}")
                    for u in range(2):
                        t_ = 2 * h + u
                        nc.tensor.matmul(
                            out=yps[:, u * NCOL:(u + 1) * NCOL],
                            lhsT=wexp_sb[32 * t_:32 * (t_ + 1), :],
                            rhs=ssb[32 * t_:32 * (t_ + 1), :],
                            start=True,
                            stop=True,
                            tile_position=(32 * t_, 0),
                        )
                    nc.scalar.activation(
                        out=fsb[:, h * 2 * NCOL:(h + 1) * 2 * NCOL],
                        in_=yps[:],
                        func=Square,
                        bias=biasw_sb[:, 0:1],
                        scale=1.0,
                    )

                ops2 = ps.tile([64, 2 * NCOL], f32)
                for ab in range(2):
                    for half in range(2):
                        t_ = 2 * half + ab
                        nc.tensor.matmul(
                            out=ops2[0:64, half * NCOL:(half + 1) * NCOL],
                            lhsT=lam_sb[:, 64 * ab:64 * (ab + 1)],
                            rhs=fsb[:, t_ * NCOL:(t_ + 1) * NCOL],
                            start=(ab == 0),
                            stop=(ab == 1),
                            tile_position=(0, 0),
                            skip_group_check=True,
                        )
                osb = sb.tile([128, NCOL], f32)
                nc.vector.tensor_copy(out=osb[0:64, :], in_=ops2[0:64, 0:NCOL])
                nc.vector.tensor_copy(
                    out=osb[64:128, :], in_=ops2[0:64, NCOL:2 * NCOL]
                )

                tps = ps.tile([128, NCOL], f32)
                for blk in range(4):
                    nc.tensor.transpose(
                        out=tps[:, blk * 128:(blk + 1) * 128],
                        in_=osb[:, blk * 128:(blk + 1) * 128],
                        identity=ident[:],
                    )
                oub = sb.tile([128, NCOL], f32)
                nc.vector.tensor_copy(out=oub[:], in_=tps[:])
                nc.sync.dma_start(out=out_r[T], in_=oub[:])

    nc.finalize()
    return nc


def _ensure_ntff_hook():
    """Install the axon NTFF profiling hook if the image's antenv lacks it."""
    import sys
    import types
    try:
        from antenv.axon_hooks import get_axon_ntff_profile_hook  # noqa: F401
        return
    except ImportError:
        pass
    try:
        import antenv
        from trn_agent_boot.trn_boot import _ntff_profile_via_ctypes
        mod = types.ModuleType("antenv.axon_hooks")
        store = [None]
        mod.set_axon_ntff_profile_hook = lambda h: store.__setitem__(0, h)
        mod.get_axon_ntff_profile_hook = lambda: store[0]
        sys.modules["antenv.axon_hooks"] = mod
        antenv.axon_hooks = mod
        mod.set_axon_ntff_profile_hook(
            _ntff_profile_via_ctypes("/opt/axon/libaxon_pjrt.so")
        )
        import concourse.bass_utils as bu
        bu.upload_artifacts = lambda tmpdir: tmpdir
    except Exception as e:  # profiling is best-effort
        print(f"ntff hook install failed: {e}")


def kernel(t, state, cmd, coeffs):
    from concourse.bass_utils import run_bass_kernel_spmd

    trace = bool(int(os.environ.get("BOAT_TRACE", "0")))
    if trace:
        _ensure_ntff_hook()

    t = np.asarray(t)
    state = np.ascontiguousarray(np.asarray(state, dtype=np.float32))
    cmd = np.asarray(cmd, dtype=np.float32)
    coeffs = np.asarray(coeffs, dtype=np.float32)

    precise = bool(int(os.environ.get("BOAT_PRECISE", "0")))
    force_j15 = bool(int(os.environ.get("BOAT_J15", "0")))

    use_j8 = False
    if not force_j15:
        Q = _build_Q(t, cmd, coeffs)
        qscale = max(np.abs(Q).max(), 1e-30)
        resid, amp, Wj8, lamj8 = _solve_j8(Q)
        use_j8 = resid < 1e-7 * qscale
        if not use_j8:
            print(f"J8 fit failed (resid {resid:.2e}, scale {qscale:.2e}); "
                  "falling back to J15 path")

    if use_j8:
        wexp, biasv, lamblk = _host_weights_j8(Wj8, lamj8)
        wexp = wexp.astype(np.float16)
        lamblk = lamblk.astype(np.float16)
        state16 = state.astype(np.float16)
        key = ("j8", precise)
        if key not in _NC_CACHE:
            _NC_CACHE[key] = _build_nc_j8(precise)
        nc = _NC_CACHE[key]
        in_maps = []
        for k in range(NCORES):
            shard = np.ascontiguousarray(state16[k * BS:(k + 1) * BS])
            in_maps.append(
                {"state": shard, "wexp": wexp, "biasv": biasv, "lam": lamblk}
            )
    else:
        wexp4, biasw, lam = _host_weights_j15(t, cmd, coeffs)
        key = ("j15", precise)
        if key not in _NC_CACHE:
            _NC_CACHE[key] = _build_nc_j15(precise)
        nc = _NC_CACHE[key]
        in_maps = []
        for k in range(NCORES):
            shard = np.ascontiguousarray(state[k * BS:(k + 1) * BS])
            in_maps.append(
                {"state": shard, "wexp": wexp4, "biasw": biasw, "lam": lam}
            )

    res = run_bass_kernel_spmd(
        nc,
        in_maps,
        core_ids=list(range(NCORES)),
        trace=trace,
    )
    LAST_RESULT[0] = res
    full = np.concatenate([r["out"] for r in res.results], axis=0)
    if full.dtype != np.float32:
        full = full.astype(np.float32)
    return full
